# revision 10
# baseline (speedup 1.0000x reference)
"""Causal self-attention (GQA + RoPE + QK-RMSNorm) on 8 trn2 NeuronCores.

Reference (B=2, T=2048, C=2048, 16 q-heads / 4 kv-heads, head_dim 128):
    q = rms_norm(rope(x @ Wq)) / sqrt(128); k = rms_norm(rope(x @ Wk))
    att = softmax_causal(q k^T / sqrt(128)); y = (att @ v) @ Wp
Sharding: core = 4*b + g  (b = batch 0..1, g = head-group 0..3).
Each core computes q-heads 4g..4g+3 (kv-head g), attends over the full
causal sequence of its batch, and produces a 512-column slice of the
output projection. Host concatenates.

Fully fused single pass over t-chunks:
  1. project chunk tc: K/Q via fp8 DoubleRow matmuls (x and Wq/Wk are
     fp8e4; weights host-scaled x64 so they clear the fp8 denormal
     range, the rms-norm eps is rescaled to compensate, and the
     normalization makes the scale cancel exactly); V in bf16.
  2. attention for the chunk's queries over keys 0..end (chunk 3 is
     split into two 256-query halves so the last AllGather overlaps
     compute); per-piece AllGather on its own DRAM tile.
  3. output projection per piece, lagged two pieces behind its
     AllGather (one 16-head PSUM accumulation, no spill-add).

Row-sum of exp'd scores: score-group halves folded on DVE, one rs
matmul per group.  Queue discipline: gpsimd = bcasts + yT stores + AG
triggers + half the x loads, sync = everything else DMA, scalar =
activations only.
"""

import ml_dtypes
import numpy as np

B, T, C = 2, 2048, 2048
NH, NKV, HD = 16, 4, 128
G = 4  # q-heads per core
EPS = 1e-6
NCB = C // 128  # 16 contraction blocks
NTCH = T // 512  # 4 t-chunks
W8SCALE = 64.0  # host premultiplier on Wq/Wk before fp8 cast

# (tc, qoff, width) attention/projection pieces; chunk 3 split in half
PIECES = [(0, 0, 512), (1, 512, 512), (2, 1024, 512), (3, 1536, 256), (3, 1792, 256)]
# AllGather groups: (ag_tile_width, [(piece, col_offset)]); pieces 0+1 share
AG_GROUPS = [(1024, [(0, 0), (1, 512)]), (512, [(2, 0)]), (256, [(3, 0)]), (256, [(4, 0)])]
PIECE_AG = {p: (gi, off) for gi, (_, ps) in enumerate(AG_GROUPS) for p, off in ps}

_CACHE = {}


def _build():
    import concourse.mybir as mybir
    import concourse.tile as tile
    from concourse import bacc
    from concourse.masks import make_identity
    from contextlib import ExitStack

    F32 = mybir.dt.float32
    BF16 = mybir.dt.bfloat16
    FP8 = mybir.dt.float8e4
    AF = mybir.ActivationFunctionType
    DR = mybir.MatmulPerfMode.DoubleRow

    nc = bacc.Bacc(None, target_bir_lowering=False, num_devices=8)

    xT = nc.dram_tensor("xT", [C, T], BF16, kind="ExternalInput")
    xT8 = nc.dram_tensor("xT8", [C, T], FP8, kind="ExternalInput")
    wq8 = nc.dram_tensor("wq8", [C, G * HD], FP8, kind="ExternalInput")
    wk8 = nc.dram_tensor("wk8", [C, HD], FP8, kind="ExternalInput")
    wv = nc.dram_tensor("wv", [C, HD], BF16, kind="ExternalInput")
    wp = nc.dram_tensor("wp", [C, G * HD], BF16, kind="ExternalInput")
    cosT = nc.dram_tensor("cosT", [128, T], BF16, kind="ExternalInput")
    sinT = nc.dram_tensor("sinT", [128, T], BF16, kind="ExternalInput")
    masks = nc.dram_tensor("masks", [4, 128, 512], BF16, kind="ExternalInput")
    outT = nc.dram_tensor("outT", [G * HD, T], BF16, kind="ExternalOutput")

    with tile.TileContext(nc) as tc_ctx:
        with ExitStack() as S:
            dram = S.enter_context(tc_ctx.tile_pool(name="dram", bufs=1, space="DRAM"))
            ag_in = [
                dram.tile([4 * HD, w], BF16, name=f"ag_in_{i}")
                for i, (w, _) in enumerate(AG_GROUPS)
            ]
            ag_out = [
                dram.tile([16 * HD, w], BF16, name=f"ag_out_{i}")
                for i, (w, _) in enumerate(AG_GROUPS)
            ]
            barrier_in = dram.tile([1, 4], BF16, name="barrier_in")
            barrier_out = dram.tile([4, 4], BF16, name="barrier_out")

            consts = S.enter_context(tc_ctx.tile_pool(name="consts", bufs=1))
            ones_bf = consts.tile([128, 1], BF16)
            nc.vector.memset(ones_bf[:], 1.0)
            c2 = W8SCALE * W8SCALE
            eps_k = consts.tile([1, 1], F32)
            nc.vector.memset(eps_k[:], EPS * c2)
            eps_q = consts.tile([1, 1], F32)
            nc.vector.memset(eps_q[:], float(HD * HD) * EPS * c2)
            ident_bf = consts.tile([128, 128], BF16)
            make_identity(nc, ident_bf[:])

            wpool = S.enter_context(tc_ctx.tile_pool(name="w", bufs=1))
            wq8_sb = wpool.tile([128, 8, 2, G * HD], FP8)
            wk8_sb = wpool.tile([128, 8, 2, HD], FP8)
            wv_sb = wpool.tile([128, NCB, HD], BF16)
            wp_sb = wpool.tile([128, NCB, G * HD], BF16)
            wk8r = wk8.rearrange("(cp two p) n -> p cp two n", p=128, two=2)
            wq8r = wq8.rearrange("(cp two p) n -> p cp two n", p=128, two=2)
            wvr = wv.rearrange("(cb p) n -> p cb n", p=128)
            wpr = wp.rearrange("(cb p) n -> p cb n", p=128)
            # wk8 first: chunk-0 K projection is the first PE work.
            nc.scalar.dma_start(out=wk8_sb[:], in_=wk8r[:])

            trig = S.enter_context(tc_ctx.tile_pool(name="trig", bufs=1))
            cos_sb = trig.tile([128, T], BF16)
            sin_sb = trig.tile([128, T], BF16)
            masks_sb = trig.tile([128, 4, 512], BF16)

            acts = S.enter_context(tc_ctx.tile_pool(name="acts", bufs=1))
            qT_sb = acts.tile([128, G, T], BF16)
            kT_sb = acts.tile([128, T], BF16)
            v_sb = acts.tile([128, NCB, HD], BF16)
            yT_sb = acts.tile([128, G, T], BF16)

            xt_pool = S.enter_context(tc_ctx.tile_pool(name="xt", bufs=8))
            x8_pool = S.enter_context(tc_ctx.tile_pool(name="x8", bufs=2))
            rem_pool = S.enter_context(tc_ctx.tile_pool(name="rem", bufs=6))
            tmp = S.enter_context(tc_ctx.tile_pool(name="tmp", bufs=2))
            rowp = S.enter_context(tc_ctx.tile_pool(name="rowp", bufs=2))
            pt_pool = S.enter_context(tc_ctx.tile_pool(name="pt", bufs=3))
            pairs_pool = S.enter_context(tc_ctx.tile_pool(name="pairs", bufs=3))
            osb_pool = S.enter_context(tc_ctx.tile_pool(name="osb", bufs=3))

            # PSUM: acc 3 + sp 2x2 + rows 1 = 8 banks
            acc = S.enter_context(tc_ctx.tile_pool(name="acc", bufs=3, space="PSUM"))
            spp = S.enter_context(tc_ctx.tile_pool(name="spp", bufs=2, space="PSUM"))
            rows = S.enter_context(tc_ctx.tile_pool(name="rows", bufs=1, space="PSUM"))

            def rope_norm(dst, psrc, tcs, sqrt_scale, sqrt_bias):
                """dst = rope(psrc) / sqrt(sqrt_scale*ssq + bias), bf16 math."""
                xb = tmp.tile([128, 512], BF16, tag="xb")
                nc.vector.tensor_copy(out=xb[:], in_=psrc)
                rot = tmp.tile([128, 512], BF16, tag="rot")
                # sin_sb rows 0-63 hold +sin, rows 64-127 hold -sin, so each
                # tensor_tensor reads both SBUF operands at the same base
                # partition (compiler constraint NCC_IBIR297).
                nc.vector.tensor_mul(rot[0:64, :], xb[64:128, :], sin_sb[64:128, tcs])
                nc.vector.tensor_mul(rot[64:128, :], xb[0:64, :], sin_sb[0:64, tcs])
                xc = tmp.tile([128, 512], BF16, tag="xc")
                nc.vector.tensor_mul(xc[:], xb[:], cos_sb[:, tcs])
                ro = tmp.tile([128, 512], BF16, tag="ro")
                nc.vector.tensor_add(ro[:], xc[:], rot[:])
                sq = tmp.tile([128, 512], BF16, tag="sq")
                nc.vector.tensor_mul(sq[:], ro[:], ro[:])
                ps_ss = rows.tile([1, 512], F32, tag="rows")
                nc.tensor.matmul(ps_ss[:], ones_bf[:], sq[:], start=True, stop=True)
                srow = rowp.tile([1, 512], F32, tag="srow")
                nc.scalar.activation(
                    out=srow[:], in_=ps_ss[:], func=AF.Sqrt,
                    scale=sqrt_scale, bias=sqrt_bias,
                )
                rrow = rowp.tile([1, 512], F32, tag="rrow")
                nc.vector.reciprocal_approx_fast(out=rrow[:], in_=srow[:])
                bc = tmp.tile([128, 512], F32, tag="bc")
                nc.gpsimd.partition_broadcast(bc[:], rrow[:])
                nc.vector.tensor_mul(dst, ro[:], bc[:])

            def load_x_chunk(tcn, first):
                """Prefetch x chunk tcn (bf16 for V, fp8 for Q/K)."""
                tcs = slice(512 * tcn, 512 * tcn + 512)
                x8t = x8_pool.tile([128, NCB, 512], FP8, tag="x8")
                x8r = xT8[:, tcs].rearrange("(cb p) t -> p cb t", p=128)
                xts = []
                if first:
                    # startup-critical: fp8 (K/Q inputs) first, fine-grained
                    engs = [nc.sync, nc.gpsimd, nc.scalar]
                    for j in range(8):
                        engs[j % 3].dma_start(
                            out=x8t[:, 2 * j : 2 * j + 2, :],
                            in_=x8r[:, 2 * j : 2 * j + 2, :],
                        )
                    for gx in range(4):
                        xt = xt_pool.tile([128, 4, 512], BF16, tag="xt")
                        xr = xT[512 * gx : 512 * (gx + 1), tcs].rearrange(
                            "(cb p) t -> p cb t", p=128
                        )
                        engs[gx % 3].dma_start(out=xt[:, 0:2, :], in_=xr[:, 0:2, :])
                        engs[(gx + 1) % 3].dma_start(
                            out=xt[:, 2:4, :], in_=xr[:, 2:4, :]
                        )
                        xts.append(xt)
                else:
                    nc.sync.dma_start(out=x8t[:, 0:8, :], in_=x8r[:, 0:8, :])
                    nc.sync.dma_start(out=x8t[:, 8:16, :], in_=x8r[:, 8:16, :])
                    for gx in range(4):
                        xt = xt_pool.tile([128, 4, 512], BF16, tag="xt")
                        xr = xT[512 * gx : 512 * (gx + 1), tcs].rearrange(
                            "(cb p) t -> p cb t", p=128
                        )
                        eng = nc.sync if gx < 2 else nc.gpsimd
                        eng.dma_start(out=xt[:], in_=xr[:])
                        xts.append(xt)
                return x8t, xts

            def proj_chunk(tcn, x8t, xts):
                """Q/K (fp8 DoubleRow) and V (bf16) projections for chunk."""
                tcs = slice(512 * tcn, 512 * tcn + 512)

                ps_k = acc.tile([128, 512], F32, tag="acc")
                for cp in range(8):
                    nc.tensor.matmul(
                        ps_k[:], wk8_sb[:, cp, :, :], x8t[:, 2 * cp : 2 * cp + 2, :],
                        start=(cp == 0), stop=(cp == 7), perf_mode=DR,
                    )
                rope_norm(kT_sb[:, tcs], ps_k[:], tcs, 1.0 / HD, eps_k[:])

                ps_v = acc.tile([128, 512], F32, tag="acc")
                for cb in range(NCB):
                    nc.tensor.matmul(
                        ps_v[:], wv_sb[:, cb, :], xts[cb // 4][:, cb % 4, :],
                        start=(cb == 0), stop=(cb == NCB - 1),
                    )
                vb = tmp.tile([128, 512], BF16, tag="vb")
                nc.vector.tensor_copy(out=vb[:], in_=ps_v[:])
                ps_tr = acc.tile([128, 512], BF16, tag="acc")
                for tt in range(4):
                    nc.tensor.transpose(
                        ps_tr[:, 128 * tt : 128 * (tt + 1)],
                        vb[:, 128 * tt : 128 * (tt + 1)],
                        ident_bf[:],
                    )
                for tt in range(4):
                    nc.vector.tensor_copy(
                        out=v_sb[:, 4 * tcn + tt, :],
                        in_=ps_tr[:, 128 * tt : 128 * (tt + 1)],
                    )

                for hq in range(G):
                    ps_q = acc.tile([128, 512], F32, tag="acc")
                    for cp in range(8):
                        nc.tensor.matmul(
                            ps_q[:],
                            wq8_sb[:, cp, :, 128 * hq : 128 * (hq + 1)],
                            x8t[:, 2 * cp : 2 * cp + 2, :],
                            start=(cp == 0), stop=(cp == 7), perf_mode=DR,
                        )
                    rope_norm(qT_sb[:, hq, tcs], ps_q[:], tcs, float(HD), eps_q[:])

            def attention_piece(pidx):
                """Attention for PIECES[pidx], all G heads; stores yT and
                fires the piece's AllGather."""
                tcn, qoff, w = PIECES[pidx]
                qs = slice(qoff, qoff + w)
                nblk = (qoff + w) // 128
                gsz = 1024 // w
                ngr = (nblk + gsz - 1) // gsz
                for hq in range(G):
                    ps_y = acc.tile([128, 512], F32, tag="acc")
                    rs = rows.tile([1, 512], F32, tag="rows")
                    for gi in range(ngr):
                        blks = list(range(gsz * gi, min(gsz * gi + gsz, nblk)))
                        nb = len(blks)
                        sp = spp.tile([128, 1024], F32, tag="sp")
                        for i, tkb in enumerate(blks):
                            nc.tensor.matmul(
                                sp[:, w * i : w * (i + 1)],
                                kT_sb[:, 128 * tkb : 128 * (tkb + 1)],
                                qT_sb[:, hq, qs],
                                start=True, stop=True,
                            )
                        pT = pt_pool.tile([128, 1024], BF16, tag="pt")
                        nc.scalar.activation(
                            out=pT[:, 0 : w * nb], in_=sp[:, 0 : w * nb], func=AF.Exp
                        )
                        for i, tkb in enumerate(blks):
                            d = tkb - qoff // 128
                            if d >= 0:
                                nc.vector.tensor_mul(
                                    pT[:, w * i : w * (i + 1)],
                                    pT[:, w * i : w * (i + 1)],
                                    masks_sb[:, d, 0:w],
                                )
                        # fold group to one [128, w] tile on DVE, then a
                        # single rs matmul per group
                        folds = [pT[:, w * i : w * (i + 1)] for i in range(nb)]
                        while len(folds) > 1:
                            nxt = []
                            for j in range(0, len(folds) - 1, 2):
                                f = pairs_pool.tile([128, 512], BF16, tag="pms")
                                nc.vector.tensor_add(
                                    f[:, 0:w], folds[j], folds[j + 1]
                                )
                                nxt.append(f[:, 0:w])
                            if len(folds) % 2:
                                nxt.append(folds[-1])
                            folds = nxt
                        nc.tensor.matmul(
                            rs[:, 0:w], ones_bf[:], folds[0],
                            start=(gi == 0), stop=(gi == ngr - 1),
                        )
                        for i, tkb in enumerate(blks):
                            nc.tensor.matmul(
                                ps_y[:, 0:w], v_sb[:, tkb, :],
                                pT[:, w * i : w * (i + 1)],
                                start=(tkb == 0), stop=(tkb == nblk - 1),
                            )
                    rrow = rowp.tile([1, 512], F32, tag="rrow2")
                    nc.vector.reciprocal_approx_fast(out=rrow[:, 0:w], in_=rs[:, 0:w])
                    bc = tmp.tile([128, 512], F32, tag="bc2")
                    nc.gpsimd.partition_broadcast(bc[:, 0:w], rrow[:, 0:w])
                    yt = yT_sb[:, hq, qs]
                    nc.vector.tensor_mul(yt, ps_y[:, 0:w], bc[:, 0:w])
                    gi, coff = PIECE_AG[pidx]
                    nc.gpsimd.dma_start(
                        out=ag_in[gi][128 * hq : 128 * (hq + 1), coff : coff + w],
                        in_=yt,
                    )
                gi, coff = PIECE_AG[pidx]
                if pidx == AG_GROUPS[gi][1][-1][0]:
                    nc.gpsimd.collective_compute(
                        "AllGather",
                        mybir.AluOpType.bypass,
                        replica_groups=[[0, 1, 2, 3], [4, 5, 6, 7]],
                        ins=[ag_in[gi][:]],
                        outs=[ag_out[gi][:]],
                    )

            def load_rem(pidx):
                """Prefetch the AllGathered heads for PIECES[pidx]."""
                _, _, w = PIECES[pidx]
                gi, coff = PIECE_AG[pidx]
                rem = []
                for s in range(4):
                    rt = rem_pool.tile(
                        [128, 4, 512], BF16, tag="rem", name=f"rem{pidx}_{s}"
                    )
                    nc.sync.dma_start(
                        out=rt[:, :, 0:w],
                        in_=ag_out[gi][
                            512 * s : 512 * (s + 1), coff : coff + w
                        ].rearrange("(b p) t -> p b t", p=128),
                    )
                    rem.append(rt)
                return rem

            def proj_out(pidx, rem):
                """Output projection for PIECES[pidx] using all 16 heads."""
                _, qoff, w = PIECES[pidx]
                for cob in range(4):
                    ps_o = acc.tile([128, 512], F32, tag="acc")
                    for r in range(NCB):
                        nc.tensor.matmul(
                            ps_o[:, 0:w],
                            wp_sb[:, r, 128 * cob : 128 * (cob + 1)],
                            rem[r // 4][:, r % 4, 0:w],
                            start=(r == 0), stop=(r == NCB - 1),
                        )
                    o_sb = osb_pool.tile([128, 512], BF16, tag="osb")
                    nc.vector.tensor_copy(out=o_sb[:, 0:w], in_=ps_o[:, 0:w])
                    nc.sync.dma_start(
                        out=outT[128 * cob : 128 * (cob + 1), qoff : qoff + w],
                        in_=o_sb[:, 0:w],
                    )

            # ---- main fused schedule ----
            # Start-alignment barrier: a tiny AllGather consumed by a dummy
            # DMA on the gpsimd queue.  Launch skew between cores is then
            # absorbed once here (while startup DMAs run) instead of
            # compounding at every mid-kernel collective.
            nc.gpsimd.collective_compute(
                "AllGather",
                mybir.AluOpType.bypass,
                replica_groups=[[0, 1, 2, 3], [4, 5, 6, 7]],
                ins=[barrier_in[:]],
                outs=[barrier_out[:]],
            )
            barrier_sb = consts.tile([1, 4], BF16)
            nc.gpsimd.dma_start(out=barrier_sb[:], in_=barrier_out[0:1, :])
            x8_cur, xts_cur = load_x_chunk(0, first=True)
            # remaining startup loads, ordered by first use
            nc.scalar.dma_start(out=cos_sb[:], in_=cosT[:])
            nc.scalar.dma_start(out=sin_sb[:], in_=sinT[:])
            nc.scalar.dma_start(out=wq8_sb[:], in_=wq8r[:])
            for i in range(2):
                cbs = slice(8 * i, 8 * i + 8)
                nc.sync.dma_start(out=wv_sb[:, cbs, :], in_=wvr[:, cbs, :])
            nc.sync.dma_start(out=masks_sb[:], in_=masks.rearrange("d p m -> p d m"))

            for tcn in range(NTCH):
                if tcn + 1 < NTCH:
                    x8_next, xts_next = load_x_chunk(tcn + 1, first=False)
                if tcn == 0:
                    # wp needed first at proj_out(piece 0), during iteration 3
                    for i in range(4):
                        cbs = slice(4 * i, 4 * i + 4)
                        nc.sync.dma_start(out=wp_sb[:, cbs, :], in_=wpr[:, cbs, :])
                proj_chunk(tcn, x8_cur, xts_cur)
                if tcn < 3:
                    attention_piece(tcn)
                else:
                    attention_piece(3)
                    proj_out(0, load_rem(0))
                    attention_piece(4)
                    proj_out(1, load_rem(1))
                if tcn + 1 < NTCH:
                    x8_cur, xts_cur = x8_next, xts_next
            proj_out(2, load_rem(2))
            proj_out(3, load_rem(3))
            proj_out(4, load_rem(4))

    nc.compile()
    return nc


def _get_nc():
    if "nc" not in _CACHE:
        _CACHE["nc"] = _build()
    return _CACHE["nc"]


def _host_inputs(x, cos, sin, Wq, Wk, Wv, Wp):
    bf16 = ml_dtypes.bfloat16
    fp8 = ml_dtypes.float8_e4m3fn
    x = np.asarray(x)
    cos = np.asarray(cos, dtype=np.float32)
    sin = np.asarray(sin, dtype=np.float32)
    cosT = np.ascontiguousarray(np.concatenate([cos.T, cos.T], axis=0)).astype(bf16)
    sinT = np.ascontiguousarray(np.concatenate([sin.T, -sin.T], axis=0)).astype(bf16)
    p = np.arange(128)[:, None]
    j = np.arange(512)[None, :]
    masks = np.stack([(j >= p + 128 * d) for d in range(4)], axis=0).astype(bf16)

    in_maps = []
    for core in range(8):
        b, g = core // 4, core % 4
        xTb = np.ascontiguousarray(np.asarray(x)[b].T)
        in_maps.append(
            {
                "xT": xTb.astype(bf16),
                "xT8": xTb.astype(fp8),
                "wq8": np.ascontiguousarray(
                    Wq[:, 512 * g : 512 * g + 512] * W8SCALE
                ).astype(fp8),
                "wk8": np.ascontiguousarray(
                    Wk[:, 128 * g : 128 * g + 128] * W8SCALE
                ).astype(fp8),
                "wv": np.ascontiguousarray(
                    Wv[:, 128 * g : 128 * g + 128]
                ).astype(bf16),
                "wp": np.ascontiguousarray(
                    Wp[:, 512 * g : 512 * g + 512]
                ).astype(bf16),
                "cosT": cosT,
                "sinT": sinT,
                "masks": masks,
            }
        )
    return in_maps


def kernel(x, cos, sin, Wq, Wk, Wv, Wp):
    from concourse.bass_utils import run_bass_kernel_spmd

    in_maps = _host_inputs(x, cos, sin, Wq, Wk, Wv, Wp)
    nc = _get_nc()
    res = run_bass_kernel_spmd(nc, in_maps, core_ids=list(range(8)), trace=False)

    out = np.empty((B, T, C), dtype=np.float32)
    for core in range(8):
        b, g = core // 4, core % 4
        out[b, :, 512 * g : 512 * g + 512] = (
            res.results[core]["outT"].T.astype(np.float32)
        )
    return out


# revision 11
# speedup vs baseline: 1.2937x; 1.2937x over previous
"""Causal self-attention (GQA + RoPE + QK-RMSNorm) on 8 trn2 NeuronCores.

Reference (B=2, T=2048, C=2048, 16 q-heads / 4 kv-heads, head_dim 128):
    q = rms_norm(rope(x @ Wq)) / sqrt(128); k = rms_norm(rope(x @ Wk))
    att = softmax_causal(q k^T / sqrt(128)); y = (att @ v) @ Wp
Sharding: core = 4*b + g  (b = batch 0..1, g = head-group 0..3).
Each core computes q-heads 4g..4g+3 (kv-head g), attends over the full
causal sequence of its batch, and produces a 512-column slice of the
output projection. Host concatenates.

Fully fused single pass over t-chunks:
  1. project chunk tc: K/Q via fp8 DoubleRow matmuls (x and Wq/Wk are
     fp8e4; weights host-scaled x64 so they clear the fp8 denormal
     range, the rms-norm eps is rescaled to compensate, and the
     normalization makes the scale cancel exactly); V in bf16.
  2. attention for the chunk's queries over keys 0..end (chunk 3 is
     split into two 256-query halves so the last AllGather overlaps
     compute); per-piece AllGather on its own DRAM tile.
  3. output projection per piece, lagged two pieces behind its
     AllGather (one 16-head PSUM accumulation, no spill-add).

Row-sum of exp'd scores: score-group halves folded on DVE, one rs
matmul per group.  Queue discipline: gpsimd = bcasts + yT stores + AG
triggers + half the x loads, sync = everything else DMA, scalar =
activations only.
"""

import ml_dtypes
import numpy as np

B, T, C = 2, 2048, 2048
NH, NKV, HD = 16, 4, 128
G = 4  # q-heads per core
EPS = 1e-6
NCB = C // 128  # 16 contraction blocks
NTCH = T // 512  # 4 t-chunks
W8SCALE = 64.0  # host premultiplier on Wq/Wk before fp8 cast

# (tc, qoff, width) attention/projection pieces; chunk 3 split in half
PIECES = [(0, 0, 512), (1, 512, 512), (2, 1024, 512), (3, 1536, 256), (3, 1792, 256)]
# AllGather groups: (ag_tile_width, [(piece, col_offset)]); pieces 0+1 share
AG_GROUPS = [(1024, [(0, 0), (1, 512)]), (512, [(2, 0)]), (256, [(3, 0)]), (256, [(4, 0)])]
PIECE_AG = {p: (gi, off) for gi, (_, ps) in enumerate(AG_GROUPS) for p, off in ps}

_CACHE = {}


def _build():
    import concourse.mybir as mybir
    import concourse.tile as tile
    from concourse import bacc
    from concourse.masks import make_identity
    from contextlib import ExitStack

    F32 = mybir.dt.float32
    BF16 = mybir.dt.bfloat16
    FP8 = mybir.dt.float8e4
    AF = mybir.ActivationFunctionType
    DR = mybir.MatmulPerfMode.DoubleRow

    nc = bacc.Bacc(None, target_bir_lowering=False, num_devices=8)

    xT = nc.dram_tensor("xT", [C, T], BF16, kind="ExternalInput")
    xT8 = nc.dram_tensor("xT8", [C, T], FP8, kind="ExternalInput")
    wq8 = nc.dram_tensor("wq8", [C, G * HD], FP8, kind="ExternalInput")
    wk8 = nc.dram_tensor("wk8", [C, HD], FP8, kind="ExternalInput")
    wv = nc.dram_tensor("wv", [C, HD], BF16, kind="ExternalInput")
    wp = nc.dram_tensor("wp", [C, G * HD], BF16, kind="ExternalInput")
    cosT = nc.dram_tensor("cosT", [128, T], BF16, kind="ExternalInput")
    sinT = nc.dram_tensor("sinT", [128, T], BF16, kind="ExternalInput")
    masks = nc.dram_tensor("masks", [4, 128, 512], BF16, kind="ExternalInput")
    outT = nc.dram_tensor("outT", [G * HD, T], BF16, kind="ExternalOutput")

    with tile.TileContext(nc) as tc_ctx:
        with ExitStack() as S:
            dram = S.enter_context(tc_ctx.tile_pool(name="dram", bufs=1, space="DRAM"))
            ag_in = [
                dram.tile([4 * HD, w], BF16, name=f"ag_in_{i}")
                for i, (w, _) in enumerate(AG_GROUPS)
            ]
            ag_out = [
                dram.tile([16 * HD, w], BF16, name=f"ag_out_{i}")
                for i, (w, _) in enumerate(AG_GROUPS)
            ]

            consts = S.enter_context(tc_ctx.tile_pool(name="consts", bufs=1))
            ones_bf = consts.tile([128, 1], BF16)
            nc.vector.memset(ones_bf[:], 1.0)
            c2 = W8SCALE * W8SCALE
            eps_k = consts.tile([1, 1], F32)
            nc.vector.memset(eps_k[:], EPS * c2)
            eps_q = consts.tile([1, 1], F32)
            nc.vector.memset(eps_q[:], float(HD * HD) * EPS * c2)
            ident_bf = consts.tile([128, 128], BF16)
            make_identity(nc, ident_bf[:])

            wpool = S.enter_context(tc_ctx.tile_pool(name="w", bufs=1))
            wq8_sb = wpool.tile([128, 8, 2, G * HD], FP8)
            wk8_sb = wpool.tile([128, 8, 2, HD], FP8)
            wv_sb = wpool.tile([128, NCB, HD], BF16)
            wp_sb = wpool.tile([128, NCB, G * HD], BF16)
            wk8r = wk8.rearrange("(cp two p) n -> p cp two n", p=128, two=2)
            wq8r = wq8.rearrange("(cp two p) n -> p cp two n", p=128, two=2)
            wvr = wv.rearrange("(cb p) n -> p cb n", p=128)
            wpr = wp.rearrange("(cb p) n -> p cb n", p=128)
            # wk8 first: chunk-0 K projection is the first PE work.
            nc.scalar.dma_start(out=wk8_sb[:], in_=wk8r[:])

            trig = S.enter_context(tc_ctx.tile_pool(name="trig", bufs=1))
            cos_sb = trig.tile([128, T], BF16)
            sin_sb = trig.tile([128, T], BF16)
            masks_sb = trig.tile([128, 4, 512], BF16)

            acts = S.enter_context(tc_ctx.tile_pool(name="acts", bufs=1))
            qT_sb = acts.tile([128, G, T], BF16)
            kT_sb = acts.tile([128, T], BF16)
            v_sb = acts.tile([128, NCB, HD], BF16)
            yT_sb = acts.tile([128, G, T], BF16)

            xt_pool = S.enter_context(tc_ctx.tile_pool(name="xt", bufs=8))
            x8_pool = S.enter_context(tc_ctx.tile_pool(name="x8", bufs=2))
            rem_pool = S.enter_context(tc_ctx.tile_pool(name="rem", bufs=6))
            tmp = S.enter_context(tc_ctx.tile_pool(name="tmp", bufs=2))
            rowp = S.enter_context(tc_ctx.tile_pool(name="rowp", bufs=2))
            pt_pool = S.enter_context(tc_ctx.tile_pool(name="pt", bufs=3))
            pairs_pool = S.enter_context(tc_ctx.tile_pool(name="pairs", bufs=3))
            osb_pool = S.enter_context(tc_ctx.tile_pool(name="osb", bufs=3))

            # PSUM: acc 3 + sp 2x2 + rows 1 = 8 banks
            acc = S.enter_context(tc_ctx.tile_pool(name="acc", bufs=3, space="PSUM"))
            spp = S.enter_context(tc_ctx.tile_pool(name="spp", bufs=2, space="PSUM"))
            rows = S.enter_context(tc_ctx.tile_pool(name="rows", bufs=1, space="PSUM"))

            def rope_norm(dst, psrc, tcs, sqrt_scale, sqrt_bias):
                """dst = rope(psrc) / sqrt(sqrt_scale*ssq + bias), bf16 math."""
                xb = tmp.tile([128, 512], BF16, tag="xb")
                nc.vector.tensor_copy(out=xb[:], in_=psrc)
                rot = tmp.tile([128, 512], BF16, tag="rot")
                # sin_sb rows 0-63 hold +sin, rows 64-127 hold -sin, so each
                # tensor_tensor reads both SBUF operands at the same base
                # partition (compiler constraint NCC_IBIR297).
                nc.vector.tensor_mul(rot[0:64, :], xb[64:128, :], sin_sb[64:128, tcs])
                nc.vector.tensor_mul(rot[64:128, :], xb[0:64, :], sin_sb[0:64, tcs])
                xc = tmp.tile([128, 512], BF16, tag="xc")
                nc.vector.tensor_mul(xc[:], xb[:], cos_sb[:, tcs])
                ro = tmp.tile([128, 512], BF16, tag="ro")
                nc.vector.tensor_add(ro[:], xc[:], rot[:])
                sq = tmp.tile([128, 512], BF16, tag="sq")
                nc.vector.tensor_mul(sq[:], ro[:], ro[:])
                ps_ss = rows.tile([1, 512], F32, tag="rows")
                nc.tensor.matmul(ps_ss[:], ones_bf[:], sq[:], start=True, stop=True)
                srow = rowp.tile([1, 512], F32, tag="srow")
                nc.scalar.activation(
                    out=srow[:], in_=ps_ss[:], func=AF.Sqrt,
                    scale=sqrt_scale, bias=sqrt_bias,
                )
                rrow = rowp.tile([1, 512], F32, tag="rrow")
                nc.vector.reciprocal_approx_fast(out=rrow[:], in_=srow[:])
                bc = tmp.tile([128, 512], F32, tag="bc")
                nc.gpsimd.partition_broadcast(bc[:], rrow[:])
                nc.vector.tensor_mul(dst, ro[:], bc[:])

            def load_x_chunk(tcn, first):
                """Prefetch x chunk tcn (bf16 for V, fp8 for Q/K)."""
                tcs = slice(512 * tcn, 512 * tcn + 512)
                x8t = x8_pool.tile([128, NCB, 512], FP8, tag="x8")
                x8r = xT8[:, tcs].rearrange("(cb p) t -> p cb t", p=128)
                xts = []
                if first:
                    # startup-critical: fp8 (K/Q inputs) first, fine-grained
                    engs = [nc.sync, nc.gpsimd, nc.scalar]
                    for j in range(8):
                        engs[j % 3].dma_start(
                            out=x8t[:, 2 * j : 2 * j + 2, :],
                            in_=x8r[:, 2 * j : 2 * j + 2, :],
                        )
                    for gx in range(4):
                        xt = xt_pool.tile([128, 4, 512], BF16, tag="xt")
                        xr = xT[512 * gx : 512 * (gx + 1), tcs].rearrange(
                            "(cb p) t -> p cb t", p=128
                        )
                        engs[gx % 3].dma_start(out=xt[:, 0:2, :], in_=xr[:, 0:2, :])
                        engs[(gx + 1) % 3].dma_start(
                            out=xt[:, 2:4, :], in_=xr[:, 2:4, :]
                        )
                        xts.append(xt)
                else:
                    nc.sync.dma_start(out=x8t[:, 0:8, :], in_=x8r[:, 0:8, :])
                    nc.sync.dma_start(out=x8t[:, 8:16, :], in_=x8r[:, 8:16, :])
                    for gx in range(4):
                        xt = xt_pool.tile([128, 4, 512], BF16, tag="xt")
                        xr = xT[512 * gx : 512 * (gx + 1), tcs].rearrange(
                            "(cb p) t -> p cb t", p=128
                        )
                        eng = nc.sync if gx < 2 else nc.gpsimd
                        eng.dma_start(out=xt[:], in_=xr[:])
                        xts.append(xt)
                return x8t, xts

            def proj_chunk(tcn, x8t, xts):
                """Q/K (fp8 DoubleRow) and V (bf16) projections for chunk."""
                tcs = slice(512 * tcn, 512 * tcn + 512)

                ps_k = acc.tile([128, 512], F32, tag="acc")
                for cp in range(8):
                    nc.tensor.matmul(
                        ps_k[:], wk8_sb[:, cp, :, :], x8t[:, 2 * cp : 2 * cp + 2, :],
                        start=(cp == 0), stop=(cp == 7), perf_mode=DR,
                    )
                rope_norm(kT_sb[:, tcs], ps_k[:], tcs, 1.0 / HD, eps_k[:])

                ps_v = acc.tile([128, 512], F32, tag="acc")
                for cb in range(NCB):
                    nc.tensor.matmul(
                        ps_v[:], wv_sb[:, cb, :], xts[cb // 4][:, cb % 4, :],
                        start=(cb == 0), stop=(cb == NCB - 1),
                    )
                vb = tmp.tile([128, 512], BF16, tag="vb")
                nc.vector.tensor_copy(out=vb[:], in_=ps_v[:])
                ps_tr = acc.tile([128, 512], BF16, tag="acc")
                for tt in range(4):
                    nc.tensor.transpose(
                        ps_tr[:, 128 * tt : 128 * (tt + 1)],
                        vb[:, 128 * tt : 128 * (tt + 1)],
                        ident_bf[:],
                    )
                for tt in range(4):
                    nc.vector.tensor_copy(
                        out=v_sb[:, 4 * tcn + tt, :],
                        in_=ps_tr[:, 128 * tt : 128 * (tt + 1)],
                    )

                for hq in range(G):
                    ps_q = acc.tile([128, 512], F32, tag="acc")
                    for cp in range(8):
                        nc.tensor.matmul(
                            ps_q[:],
                            wq8_sb[:, cp, :, 128 * hq : 128 * (hq + 1)],
                            x8t[:, 2 * cp : 2 * cp + 2, :],
                            start=(cp == 0), stop=(cp == 7), perf_mode=DR,
                        )
                    rope_norm(qT_sb[:, hq, tcs], ps_q[:], tcs, float(HD), eps_q[:])

            def attention_piece(pidx):
                """Attention for PIECES[pidx], all G heads; stores yT and
                fires the piece's AllGather."""
                tcn, qoff, w = PIECES[pidx]
                qs = slice(qoff, qoff + w)
                nblk = (qoff + w) // 128
                gsz = 1024 // w
                ngr = (nblk + gsz - 1) // gsz
                for hq in range(G):
                    ps_y = acc.tile([128, 512], F32, tag="acc")
                    rs = rows.tile([1, 512], F32, tag="rows")
                    for gi in range(ngr):
                        blks = list(range(gsz * gi, min(gsz * gi + gsz, nblk)))
                        nb = len(blks)
                        sp = spp.tile([128, 1024], F32, tag="sp")
                        for i, tkb in enumerate(blks):
                            nc.tensor.matmul(
                                sp[:, w * i : w * (i + 1)],
                                kT_sb[:, 128 * tkb : 128 * (tkb + 1)],
                                qT_sb[:, hq, qs],
                                start=True, stop=True,
                            )
                        pT = pt_pool.tile([128, 1024], BF16, tag="pt")
                        nc.scalar.activation(
                            out=pT[:, 0 : w * nb], in_=sp[:, 0 : w * nb], func=AF.Exp
                        )
                        for i, tkb in enumerate(blks):
                            d = tkb - qoff // 128
                            if d >= 0:
                                nc.vector.tensor_mul(
                                    pT[:, w * i : w * (i + 1)],
                                    pT[:, w * i : w * (i + 1)],
                                    masks_sb[:, d, 0:w],
                                )
                        # fold group to one [128, w] tile on DVE, then a
                        # single rs matmul per group
                        folds = [pT[:, w * i : w * (i + 1)] for i in range(nb)]
                        while len(folds) > 1:
                            nxt = []
                            for j in range(0, len(folds) - 1, 2):
                                f = pairs_pool.tile([128, 512], BF16, tag="pms")
                                nc.vector.tensor_add(
                                    f[:, 0:w], folds[j], folds[j + 1]
                                )
                                nxt.append(f[:, 0:w])
                            if len(folds) % 2:
                                nxt.append(folds[-1])
                            folds = nxt
                        nc.tensor.matmul(
                            rs[:, 0:w], ones_bf[:], folds[0],
                            start=(gi == 0), stop=(gi == ngr - 1),
                        )
                        for i, tkb in enumerate(blks):
                            nc.tensor.matmul(
                                ps_y[:, 0:w], v_sb[:, tkb, :],
                                pT[:, w * i : w * (i + 1)],
                                start=(tkb == 0), stop=(tkb == nblk - 1),
                            )
                    rrow = rowp.tile([1, 512], F32, tag="rrow2")
                    nc.vector.reciprocal_approx_fast(out=rrow[:, 0:w], in_=rs[:, 0:w])
                    bc = tmp.tile([128, 512], F32, tag="bc2")
                    nc.gpsimd.partition_broadcast(bc[:, 0:w], rrow[:, 0:w])
                    yt = yT_sb[:, hq, qs]
                    nc.vector.tensor_mul(yt, ps_y[:, 0:w], bc[:, 0:w])
                    gi, coff = PIECE_AG[pidx]
                    nc.gpsimd.dma_start(
                        out=ag_in[gi][128 * hq : 128 * (hq + 1), coff : coff + w],
                        in_=yt,
                    )
                gi, coff = PIECE_AG[pidx]
                if pidx == AG_GROUPS[gi][1][-1][0]:
                    nc.gpsimd.collective_compute(
                        "AllGather",
                        mybir.AluOpType.bypass,
                        replica_groups=[[0, 1, 2, 3], [4, 5, 6, 7]],
                        ins=[ag_in[gi][:]],
                        outs=[ag_out[gi][:]],
                    )

            def load_rem(pidx):
                """Prefetch the AllGathered heads for PIECES[pidx]."""
                _, _, w = PIECES[pidx]
                gi, coff = PIECE_AG[pidx]
                rem = []
                for s in range(4):
                    rt = rem_pool.tile(
                        [128, 4, 512], BF16, tag="rem", name=f"rem{pidx}_{s}"
                    )
                    nc.sync.dma_start(
                        out=rt[:, :, 0:w],
                        in_=ag_out[gi][
                            512 * s : 512 * (s + 1), coff : coff + w
                        ].rearrange("(b p) t -> p b t", p=128),
                    )
                    rem.append(rt)
                return rem

            def proj_out(pidx, rem):
                """Output projection for PIECES[pidx] using all 16 heads."""
                _, qoff, w = PIECES[pidx]
                for cob in range(4):
                    ps_o = acc.tile([128, 512], F32, tag="acc")
                    for r in range(NCB):
                        nc.tensor.matmul(
                            ps_o[:, 0:w],
                            wp_sb[:, r, 128 * cob : 128 * (cob + 1)],
                            rem[r // 4][:, r % 4, 0:w],
                            start=(r == 0), stop=(r == NCB - 1),
                        )
                    o_sb = osb_pool.tile([128, 512], BF16, tag="osb")
                    nc.vector.tensor_copy(out=o_sb[:, 0:w], in_=ps_o[:, 0:w])
                    nc.sync.dma_start(
                        out=outT[128 * cob : 128 * (cob + 1), qoff : qoff + w],
                        in_=o_sb[:, 0:w],
                    )

            # ---- main fused schedule ----
            x8_cur, xts_cur = load_x_chunk(0, first=True)
            # remaining startup loads, ordered by first use
            nc.scalar.dma_start(out=cos_sb[:], in_=cosT[:])
            nc.scalar.dma_start(out=sin_sb[:], in_=sinT[:])
            nc.scalar.dma_start(out=wq8_sb[:], in_=wq8r[:])
            for i in range(2):
                cbs = slice(8 * i, 8 * i + 8)
                nc.sync.dma_start(out=wv_sb[:, cbs, :], in_=wvr[:, cbs, :])
            nc.sync.dma_start(out=masks_sb[:], in_=masks.rearrange("d p m -> p d m"))

            for tcn in range(NTCH):
                if tcn + 1 < NTCH:
                    x8_next, xts_next = load_x_chunk(tcn + 1, first=False)
                if tcn == 0:
                    # wp needed first at proj_out(piece 0), during iteration 3
                    for i in range(4):
                        cbs = slice(4 * i, 4 * i + 4)
                        nc.sync.dma_start(out=wp_sb[:, cbs, :], in_=wpr[:, cbs, :])
                proj_chunk(tcn, x8_cur, xts_cur)
                if tcn < 3:
                    attention_piece(tcn)
                else:
                    attention_piece(3)
                    proj_out(0, load_rem(0))
                    attention_piece(4)
                    proj_out(1, load_rem(1))
                if tcn + 1 < NTCH:
                    x8_cur, xts_cur = x8_next, xts_next
            proj_out(2, load_rem(2))
            proj_out(3, load_rem(3))
            proj_out(4, load_rem(4))

    nc.compile()
    return nc


def _get_nc():
    if "nc" not in _CACHE:
        _CACHE["nc"] = _build()
    return _CACHE["nc"]


def _host_inputs(x, cos, sin, Wq, Wk, Wv, Wp):
    bf16 = ml_dtypes.bfloat16
    fp8 = ml_dtypes.float8_e4m3fn
    x = np.asarray(x)
    cos = np.asarray(cos, dtype=np.float32)
    sin = np.asarray(sin, dtype=np.float32)
    cosT = np.ascontiguousarray(np.concatenate([cos.T, cos.T], axis=0)).astype(bf16)
    sinT = np.ascontiguousarray(np.concatenate([sin.T, -sin.T], axis=0)).astype(bf16)
    p = np.arange(128)[:, None]
    j = np.arange(512)[None, :]
    masks = np.stack([(j >= p + 128 * d) for d in range(4)], axis=0).astype(bf16)

    in_maps = []
    for core in range(8):
        b, g = core // 4, core % 4
        xTb = np.ascontiguousarray(np.asarray(x)[b].T)
        in_maps.append(
            {
                "xT": xTb.astype(bf16),
                "xT8": xTb.astype(fp8),
                "wq8": np.ascontiguousarray(
                    Wq[:, 512 * g : 512 * g + 512] * W8SCALE
                ).astype(fp8),
                "wk8": np.ascontiguousarray(
                    Wk[:, 128 * g : 128 * g + 128] * W8SCALE
                ).astype(fp8),
                "wv": np.ascontiguousarray(
                    Wv[:, 128 * g : 128 * g + 128]
                ).astype(bf16),
                "wp": np.ascontiguousarray(
                    Wp[:, 512 * g : 512 * g + 512]
                ).astype(bf16),
                "cosT": cosT,
                "sinT": sinT,
                "masks": masks,
            }
        )
    return in_maps


def kernel(x, cos, sin, Wq, Wk, Wv, Wp):
    from concourse.bass_utils import run_bass_kernel_spmd

    in_maps = _host_inputs(x, cos, sin, Wq, Wk, Wv, Wp)
    nc = _get_nc()
    res = run_bass_kernel_spmd(nc, in_maps, core_ids=list(range(8)), trace=False)

    out = np.empty((B, T, C), dtype=np.float32)
    for core in range(8):
        b, g = core // 4, core % 4
        out[b, :, 512 * g : 512 * g + 512] = (
            res.results[core]["outT"].T.astype(np.float32)
        )
    return out
